# revision 4
# baseline (speedup 1.0000x reference)
"""v4: Tensor-parallel attention on 8 TRN2 cores.

Changes vs v3:
- Q/K written to DRAM in natural (token, feature) layout; attention loads
  Q^T/K^T via DMA xbar transpose (DRAM->SBUF), eliminating all PE transposes
  and their PSUM->SBUF copies in the projection.
- Weight and x tiles split into 8-d-tile sub-tiles so the first matmuls
  start after ~1.5MB of DMA instead of ~16MB.
- Freed PSUM banks -> den/ot pools double-buffered.


Changes vs v2:
- Per-batch interleaved build: proj(b0) -> attn(b0)+AG(b0) -> proj(b1) ->
  attn(b1)+AG(b1) -> wo.  Attention(b0) matmuls fill projection(b1) PE gaps,
  AllGather(b0) and wo(b0) overlap attention/projection of b1.
- PSUM split 4/4 between projection (one shared q/k/v tag ping-pong + 2
  transpose banks) and attention (2 score banks + den + ot) so both phases'
  pools coexist.
- Projection accumulates q/k/v sequentially per token tile (32 consecutive
  matmuls per group) instead of round-robin.
"""

import math
import sys

import numpy as np

sys.path.insert(0, "/opt/trn_rl_repo")

import ml_dtypes  # noqa: E402

import concourse.bass as bass  # noqa: E402,F401
import concourse.mybir as mybir  # noqa: E402
from concourse import bacc, tile  # noqa: E402
from concourse.bass_utils import run_bass_kernel_spmd  # noqa: E402
from concourse.masks import make_identity  # noqa: E402

B, S, D, H = 2, 2048, 4096, 32
HD = 128
NCORES = 8
HLOC = H // NCORES          # 4 heads per core
EL = HLOC * HD              # 512
T = B * S                   # 4096
P = 128
NT = T // P                 # 32
ND = D // P                 # 32
NB = S // P                 # 16 key blocks per batch
IC = 512                    # query-chunk width in attention
NIC = S // IC               # 4 query chunks per batch
SCALE = 1.0 / math.sqrt(HD)
BF = mybir.dt.bfloat16
F32 = mybir.dt.float32
TCH = 256
NCH_B = S // TCH            # x chunks per batch element (8)

_GRAPH_CACHE: dict[bytes, object] = {}
LAST_RESULTS = None


def _classify_mask(mask: np.ndarray):
    mb = mask.reshape(NB, P, NB, P).transpose(0, 2, 1, 3)
    kinds = np.empty((NB, NB), np.int64)
    uniq: dict[bytes, int] = {}
    blocks: list[np.ndarray] = []
    for i in range(NB):
        for j in range(NB):
            blk = mb[i, j]
            if not blk.any():
                kinds[i, j] = -2
            elif np.all(blk <= -1e8):
                kinds[i, j] = -1
            else:
                key = blk.tobytes()
                if key not in uniq:
                    uniq[key] = len(blocks)
                    blocks.append(np.ascontiguousarray(blk, np.float32))
                kinds[i, j] = uniq[key]
    return kinds, blocks


def _build(kinds: np.ndarray, n_blocks: int):
    nu = max(1, n_blocks)
    nc = bacc.Bacc("TRN2", target_bir_lowering=False, debug=False,
                   num_devices=NCORES)
    xt = nc.dram_tensor("xt", [D, T], BF, kind="ExternalInput")
    wqt = nc.dram_tensor("wqt", [D, EL], BF, kind="ExternalInput")
    wkt = nc.dram_tensor("wkt", [D, EL], BF, kind="ExternalInput")
    wvt = nc.dram_tensor("wvt", [D, EL], BF, kind="ExternalInput")
    wot = nc.dram_tensor("wot", [D, EL], BF, kind="ExternalInput")
    cosr = nc.dram_tensor("cosr", [S, EL // 2], F32, kind="ExternalInput")
    sinr = nc.dram_tensor("sinr", [S, EL // 2], F32, kind="ExternalInput")
    mblk = nc.dram_tensor("mblk", [nu, P, P], F32, kind="ExternalInput")
    out = nc.dram_tensor("out", [T, EL], F32, kind="ExternalOutput")

    HS = S // 2
    qtd = [[nc.dram_tensor(f"qnd{b}_{h}", [HS, EL], BF) for h in range(2)]
           for b in range(B)]
    ktd = [[nc.dram_tensor(f"knd{b}_{h}", [HS, EL], BF) for h in range(2)]
           for b in range(B)]
    vd = [[nc.dram_tensor(f"vd{b}_{h}", [HS, EL], BF) for h in range(2)]
          for b in range(B)]
    NCC = B * NIC
    cc_in = [nc.dram_tensor(f"cc_in{k}", [EL, IC], BF) for k in range(NCC)]
    cc_out = [nc.dram_tensor(f"cc_out{k}", [NCORES * EL, IC], BF,
                             addr_space="Shared") for k in range(NCC)]

    jlists = []
    for icq in range(NIC):
        jl = [j for j in range(NB)
              if any(kinds[4 * icq + bi, j] != -1 for bi in range(4))]
        assert jl, "fully-masked query chunk"
        jlists.append(jl)

    with tile.TileContext(nc) as tc:
        with (
            tc.tile_pool(name="const", bufs=1) as cpool,
            tc.tile_pool(name="apool", bufs=2) as apool,
            tc.tile_pool(name="ptpool", bufs=4) as ptpool,
            tc.tile_pool(name="rpool", bufs=2) as rpool,
            tc.tile_pool(name="opool", bufs=2) as opool,
            tc.tile_pool(name="spp", bufs=2, space="PSUM") as spp,
            tc.tile_pool(name="dnp", bufs=2, space="PSUM") as dnp,
            tc.tile_pool(name="otp", bufs=2, space="PSUM") as otp,
        ):
            ones_sb = cpool.tile([P, P], BF, name="ones_sb")
            nc.vector.memset(ones_sb, 1.0)
            mb_sb = cpool.tile([P, nu, P], F32, name="mb_sb")
            nc.scalar.dma_start(mb_sb, mblk.ap().rearrange("n p q -> p n q"))

            def proj_batch(b, w_parts, xpool, cspool, stg, rot, pp):
                for chb in range(NCH_B):
                    c0 = b * S + chb * TCH
                    xt_parts = []
                    for c4 in range(4):
                        xp = xpool.tile([P, 8, TCH], BF, tag=f"xt{c4}")
                        nc.sync.dma_start(
                            xp,
                            xt.ap()[c4 * 8 * P:(c4 + 1) * 8 * P,
                                    c0:c0 + TCH]
                            .rearrange("(n p) t -> p n t", p=P))
                        xt_parts.append(xp)
                    for tt in range(TCH // P):
                        s0 = chb * TCH + tt * P
                        cs_sb = cspool.tile([P, EL // 2], F32, tag="cs")
                        sn_sb = cspool.tile([P, EL // 2], F32, tag="sn")
                        nc.sync.dma_start(cs_sb, cosr.ap()[s0:s0 + P, :])
                        nc.sync.dma_start(sn_sb, sinr.ap()[s0:s0 + P, :])
                        for w_parts_i, dstd in ((0, None), (1, qtd),
                                                (2, ktd)):
                            wp = w_parts[w_parts_i]
                            ps = pp.tile([P, EL], F32, tag="pqkv")
                            for dt in range(ND):
                                nc.tensor.matmul(
                                    ps,
                                    xt_parts[dt // 8][:, dt % 8,
                                                      tt * P:(tt + 1) * P],
                                    wp[dt // 8][:, dt % 8],
                                    start=(dt == 0),
                                    stop=(dt == ND - 1))
                            h2 = s0 // HS
                            r0 = s0 % HS
                            if dstd is None:
                                v_stage = stg.tile([P, EL], BF, tag="vstg")
                                nc.any.tensor_copy(v_stage, ps)
                                nc.sync.dma_start(
                                    vd[b][h2].ap()[r0:r0 + P, :], v_stage)
                                continue
                            qn = stg.tile([P, EL], BF, tag="qn")
                            pe = ps.rearrange("p (r two) -> p r two", two=2)
                            qe = qn.rearrange("p (r two) -> p r two", two=2)
                            t1 = rot.tile([P, EL // 2], F32, tag="t1")
                            t2 = rot.tile([P, EL // 2], F32, tag="t2")
                            nc.vector.tensor_mul(t1, pe[:, :, 0], cs_sb)
                            nc.vector.tensor_mul(t2, pe[:, :, 1], sn_sb)
                            nc.vector.tensor_sub(qe[:, :, 0], t1, t2)
                            nc.vector.tensor_mul(t1, pe[:, :, 0], sn_sb)
                            nc.vector.tensor_mul(t2, pe[:, :, 1], cs_sb)
                            nc.vector.tensor_add(qe[:, :, 1], t1, t2)
                            nc.sync.dma_start(
                                dstd[b][h2].ap()[r0:r0 + P, :], qn)

            def attn_batch(b):
                for hh in range(HLOC):
                    qt_i = apool.tile([P, S], BF, tag="qt_i")
                    kt_i = apool.tile([P, S], BF, tag="kt_i")
                    v_i = apool.tile([P, NB, P], BF, tag="v_i")
                    for h2 in range(2):
                        nc.sync.dma_start_transpose(
                            qt_i[:, h2 * HS:(h2 + 1) * HS],
                            qtd[b][h2].ap()[:, hh * P:(hh + 1) * P])
                        nc.sync.dma_start_transpose(
                            kt_i[:, h2 * HS:(h2 + 1) * HS],
                            ktd[b][h2].ap()[:, hh * P:(hh + 1) * P])
                        nc.sync.dma_start(
                            v_i[:, h2 * (NB // 2):(h2 + 1) * (NB // 2), :],
                            vd[b][h2].ap()[:, hh * P:(hh + 1) * P]
                            .rearrange("(n p) e -> p n e", p=P))
                    for icq in range(NIC):
                        jl = jlists[icq]
                        den_ps = dnp.tile([P, IC], F32, tag="den_ps")
                        ot_ps = otp.tile([P, IC], F32, tag="ot_ps")
                        qslice = qt_i[:, icq * IC:(icq + 1) * IC]
                        for idx, j in enumerate(jl):
                            st = idx == 0
                            sp = idx == len(jl) - 1
                            # leading fully-masked i-sub-blocks contribute 0:
                            # narrow all ops to the live suffix (first j in
                            # jl must be full-width to init the psum group)
                            nlead = 0
                            if not st:
                                for bi in range(4):
                                    if kinds[4 * icq + bi, j] == -1:
                                        nlead += 1
                                    else:
                                        break
                            off = nlead * P
                            w = IC - off
                            sps = spp.tile([P, IC], F32, tag="sps")
                            nc.tensor.matmul(
                                sps[:, off:], kt_i[:, j * P:(j + 1) * P],
                                qslice[:, off:], start=True, stop=True)
                            for bi in range(nlead, 4):
                                k = kinds[4 * icq + bi, j]
                                if k == -1:
                                    nc.vector.memset(
                                        sps[:, bi * P:(bi + 1) * P], -1e9)
                                elif k >= 0:
                                    nc.vector.tensor_add(
                                        sps[:, bi * P:(bi + 1) * P],
                                        sps[:, bi * P:(bi + 1) * P],
                                        mb_sb[:, k, :])
                            pt = ptpool.tile([P, IC], BF, tag="pt")
                            nc.scalar.activation(
                                pt[:, off:], sps[:, off:],
                                mybir.ActivationFunctionType.Exp,
                                scale=SCALE)
                            nc.tensor.matmul(den_ps[:, off:], ones_sb,
                                             pt[:, off:], start=st, stop=sp)
                            nc.tensor.matmul(ot_ps[:, off:], v_i[:, j],
                                             pt[:, off:], start=st, stop=sp)
                        rec = rpool.tile([P, IC], F32, tag="rec")
                        nc.vector.reciprocal_approx_fast(rec, den_ps)
                        ot_sb = opool.tile([P, IC], BF, tag="ot_sb")
                        nc.vector.tensor_mul(ot_sb, ot_ps, rec)
                        nc.sync.dma_start(
                            cc_in[b * NIC + icq]
                            .ap()[hh * P:(hh + 1) * P, :], ot_sb)
                for icq in range(NIC):
                    k = b * NIC + icq
                    nc.gpsimd.collective_compute(
                        "AllGather", mybir.AluOpType.bypass,
                        ins=[cc_in[k].ap().opt()],
                        outs=[cc_out[k].ap().opt()],
                        replica_groups=[list(range(NCORES))],
                    )

            with (
                tc.tile_pool(name="wpool", bufs=1) as wpool,
                tc.tile_pool(name="xpool", bufs=2) as xpool,
                tc.tile_pool(name="cspool", bufs=2) as cspool,
                tc.tile_pool(name="stg", bufs=3) as stg,
                tc.tile_pool(name="rot", bufs=2) as rot,
                tc.tile_pool(name="pp", bufs=2, space="PSUM") as pp,
            ):
                w_parts = [[], [], []]
                for wi, w_d in ((0, wvt), (1, wqt), (2, wkt)):
                    for c4 in range(4):
                        wp = wpool.tile([P, 8, EL], BF,
                                        name=f"w{wi}_{c4}")
                        eng = nc.sync if (wi == 0 and c4 == 0) else nc.scalar
                        eng.dma_start(
                            wp,
                            w_d.ap()[c4 * 8 * P:(c4 + 1) * 8 * P, :]
                            .rearrange("(n p) e -> p n e", p=P))
                        w_parts[wi].append(wp)
                proj_batch(0, w_parts, xpool, cspool, stg, rot, pp)
                attn_batch(0)
                proj_batch(1, w_parts, xpool, cspool, stg, rot, pp)
            attn_batch(1)

            with (
                tc.tile_pool(name="wop", bufs=1) as wop,
                tc.tile_pool(name="ccp", bufs=3) as ccp,
                tc.tile_pool(name="obp", bufs=2) as obp,
                tc.tile_pool(name="wpp", bufs=2, space="PSUM") as wpp,
            ):
                wo_sb = wop.tile([P, ND, EL], BF, name="wo_sb")
                nc.sync.dma_start(
                    wo_sb, wot.ap().rearrange("(n p) e -> p n e", p=P))
                for k in range(NCC):
                    for t2 in range(IC // (2 * P)):
                        cct = ccp.tile([P, ND, 2 * P], BF, tag="cct")
                        nc.sync.dma_start(
                            cct,
                            cc_out[k].ap()[:, t2 * 2 * P:(t2 + 1) * 2 * P]
                            .rearrange("(n p) t -> p n t", p=P))
                        for tt in range(2):
                            g = k * (IC // P) + t2 * 2 + tt
                            ops = wpp.tile([P, EL], F32, tag="ops")
                            for ct in range(ND):
                                nc.tensor.matmul(
                                    ops, cct[:, ct, tt * P:(tt + 1) * P],
                                    wo_sb[:, ct],
                                    start=(ct == 0), stop=(ct == ND - 1))
                            ob = obp.tile([P, EL], F32, tag="ob")
                            nc.any.tensor_copy(ob, ops)
                            nc.sync.dma_start(
                                out.ap()[g * P:(g + 1) * P, :], ob)

    nc.compile()
    return nc


def kernel(x, wq, wk, wv, wo, freqs_cos, freqs_sin, mask, start_pos=0,
           **_ignored):
    global LAST_RESULTS
    bf = ml_dtypes.bfloat16
    mask = np.asarray(mask, np.float32)
    kinds, blocks = _classify_mask(mask)
    key = kinds.tobytes() + bytes([len(blocks)])
    nc = _GRAPH_CACHE.get(key)
    if nc is None:
        nc = _build(kinds, len(blocks))
        _GRAPH_CACHE[key] = nc

    xt_np = np.ascontiguousarray(
        np.asarray(x, np.float32).reshape(T, D).T).astype(bf)
    cos_r = np.ascontiguousarray(
        np.tile(np.asarray(freqs_cos, np.float32), (1, HLOC)))
    sin_r = np.ascontiguousarray(
        np.tile(np.asarray(freqs_sin, np.float32), (1, HLOC)))
    if blocks:
        mb_np = np.ascontiguousarray(
            np.stack([b.T for b in blocks]))  # transposed for ST layout
    else:
        mb_np = np.zeros((1, P, P), np.float32)

    in_maps = []
    for c in range(NCORES):
        hs = slice(c * HLOC, (c + 1) * HLOC)
        wq_c = np.ascontiguousarray(
            np.asarray(wq, np.float32)[hs].reshape(EL, D).T).astype(bf)
        wk_c = np.ascontiguousarray(
            np.asarray(wk, np.float32)[hs].reshape(EL, D).T).astype(bf)
        wv_c = np.ascontiguousarray(
            np.asarray(wv, np.float32)[hs].reshape(EL, D).T).astype(bf)
        wo_c = np.ascontiguousarray(
            np.asarray(wo, np.float32)[c * EL:(c + 1) * EL, :].T).astype(bf)
        in_maps.append({
            "xt": xt_np, "wqt": wq_c, "wkt": wk_c, "wvt": wv_c, "wot": wo_c,
            "cosr": cos_r, "sinr": sin_r, "mblk": mb_np,
        })

    res = run_bass_kernel_spmd(nc, in_maps, core_ids=list(range(NCORES)))
    LAST_RESULTS = res
    outs = [res.results[c]["out"] for c in range(NCORES)]
    full = np.concatenate(outs, axis=1).astype(np.float32)
    return full.reshape(B, S, D)
